# revision 11
# baseline (speedup 1.0000x reference)
"""GCN encoder (nn_GCNEncoder_74990128988468) on 8 Trainium2 NeuronCores.

Strategy (edge-parallel, per the sharding hint):
  * Sort edges by dst, shard E/8 contiguous edges per core.
  * Per core the device runs the src_fc MLP (effective 512->1024->2048->1024->128;
    the node_feat block of the input is structurally zero) over its edges in
    bf16 with fp32 PSUM accumulation, computes the Gaussian radial edge_attr on
    device, and segment-sums via a rank-compression matmul per 128-edge chunk
    (S built on device with is_equal against an iota tile; fc's Wc slice is
    folded into src_fc layer 4).
  * dst_fc has only 100 distinct inputs (embedding rows) -> computed once per
    core as a 128-wide table; feat_fc(0) and all bias paths constant-folded
    into a single per-feature bias.
  * Host does index manipulation only: sort/shard/gather + final scatter-add.
"""
import sys

sys.path.insert(0, '/opt/trn_rl_repo')

import numpy as np
import ml_dtypes

import concourse.bass as bass
import concourse.mybir as mybir
import concourse.tile as tile
from concourse.bass_utils import run_bass_kernel_spmd

F32 = np.float32
BF16 = ml_dtypes.bfloat16

N_ATOM_TYPE = 100
N_GAUSS = 128
ATOM_EMBED = 256
RADIAL = 256
HIDDEN = 1024
CUTOFF = 5.0
N_CORES = 8
P = 128
T_EDGE = 512
CPT = T_EDGE // P              # chunks per compute tile

DT_BF = mybir.dt.bfloat16
DT_F32 = mybir.dt.float32
AF = mybir.ActivationFunctionType
ALU = mybir.AluOpType
ts = bass.ts


def _nchunk_for(E):
    epc = E // N_CORES
    nch = -(-epc // P)
    return -(-nch // CPT) * CPT


# ----------------------------------------------------------------- host math
def _gelu_tanh(x):
    x = x.astype(F32)
    return F32(0.5) * x * (F32(1.0) + np.tanh(
        F32(np.sqrt(2.0 / np.pi)) * (x + F32(0.044715) * x * x * x)))


def _mlp_host(ps, x):
    x = x.astype(F32)
    for i, (W, b) in enumerate(ps):
        x = x @ np.asarray(W, F32) + np.asarray(b, F32)
        if i < len(ps) - 1:
            x = _gelu_tanh(x)
    return x


def _prep_constants(params):
    emb = np.asarray(params["atom_embedding"], F32)
    src_ps = [(np.asarray(W, F32), np.asarray(b, F32)) for W, b in params["src_fc"]]
    dst_ps = [(np.asarray(W, F32), np.asarray(b, F32)) for W, b in params["dst_fc"]]
    feat_ps = [(np.asarray(W, F32), np.asarray(b, F32)) for W, b in params["feat_fc"]]
    Wfc = np.asarray(params["fc"][0], F32)
    bfc = np.asarray(params["fc"][1], F32)
    Wa, Wb, Wc = Wfc[0:128], Wfc[128:256], Wfc[256:384]

    r = _mlp_host(feat_ps, np.zeros((1, N_GAUSS), F32))[0]   # feat_fc(0)

    # L1 split: node_feat rows (0:128) multiply zeros -> dropped; node_attr
    # rows (128:384) have only 100 distinct inputs -> precompute emb @ W1a as
    # a [100, 1024] table (gathered per edge on host); only the radial rows
    # (384:640) stay as a matmul.
    W1a = src_ps[0][0][128:384]
    W1s = src_ps[0][0][384:640]
    tab1 = (emb @ W1a).astype(BF16)                          # [100, 1024]
    W4p = src_ps[3][0] @ Wc
    b4p = src_ps[3][1] @ Wc
    W4q = dst_ps[3][0] @ Wa
    cvec = dst_ps[3][1] @ Wa + r @ Wb + bfc

    embT = np.zeros((ATOM_EMBED, P), F32)
    embT[:, :N_ATOM_TYPE] = emb.T

    mu = np.linspace(0.0, CUTOFF, RADIAL, dtype=F32)
    sigma = F32(mu[1] - mu[0])

    def bfw(a):
        return np.ascontiguousarray(a.astype(BF16))

    def btile(b, nm):  # [nm*128] -> [128, nm]
        return np.ascontiguousarray(b.reshape(nm, P).T.astype(F32))

    W = dict(
        w1s=bfw(W1s), w2s=bfw(src_ps[1][0]), w3s=bfw(src_ps[2][0]), w4p=bfw(W4p),
        b1s=btile(src_ps[0][1], 8), b2s=btile(src_ps[1][1], 16), b3s=btile(src_ps[2][1], 8),
        b4bc=np.ascontiguousarray(np.broadcast_to(b4p.astype(F32), (P, P))),
        w1d=bfw(dst_ps[0][0]), w2d=bfw(dst_ps[1][0]), w3d=bfw(dst_ps[2][0]), w4q=bfw(W4q),
        b1d=btile(dst_ps[0][1], 8), b2d=btile(dst_ps[1][1], 16), b3d=btile(dst_ps[2][1], 8),
        cvec=np.ascontiguousarray(cvec.astype(F32)[:, None]),
        embt=bfw(embT),
        mubias=np.ascontiguousarray((-(mu / sigma)).reshape(2, P).T.astype(F32)),
        ones3=np.ones((3, 1), F32),
        onesk1=np.ones((1, P), F32),
        iota=np.ascontiguousarray(np.broadcast_to(np.arange(P, dtype=F32), (P, P))),
    )
    return W, emb, sigma, tab1


def _prep_edges(atom_types, atom_coord, edge_index, tab1, nchunk):
    atom_types = np.asarray(atom_types)
    atom_coord = np.asarray(atom_coord, F32)
    src = np.asarray(edge_index[0]).astype(np.int64)
    dst = np.asarray(edge_index[1]).astype(np.int64)
    E = src.shape[0]
    epc = E // N_CORES
    epc_pad = nchunk * P

    order = np.argsort(dst, kind="stable")
    src_s, dst_s = src[order], dst[order]

    cores = []
    for c in range(N_CORES):
        s = src_s[c * epc:(c + 1) * epc]
        d = dst_s[c * epc:(c + 1) * epc]
        pad = epc_pad - epc
        s_pad = np.concatenate([s, np.full(pad, s[0], np.int64)])
        d_pad = np.concatenate([d, np.full(pad, -1, np.int64)])

        is_new = np.ones(epc_pad, bool)
        is_new[1:] = d_pad[1:] != d_pad[:-1]
        is_new[::P] = True
        rank = (np.cumsum(is_new.reshape(nchunk, P).astype(np.int64), axis=1) - 1).reshape(-1)

        real = np.arange(epc_pad) < epc
        ranks_f = np.where(real, rank, -1).astype(F32)
        ranksT = np.ascontiguousarray(ranks_f.reshape(nchunk, P).T)  # [128, nchunk]

        dstmap = np.full((nchunk, P), -1, np.int64)
        fn = is_new & real
        cid = np.arange(epc_pad) // P
        dstmap[cid[fn], rank[fn]] = d_pad[fn]

        types_src = atom_types[s_pad]
        g1t = np.ascontiguousarray(tab1[types_src].T)                # [1024, epc_pad]
        csrct = np.ascontiguousarray(atom_coord[s_pad].T)            # [3, epc_pad]
        d_safe = np.where(real, np.maximum(d_pad, 0), s_pad)
        cdstt = np.ascontiguousarray(atom_coord[d_safe].T)

        cores.append(dict(g1t=g1t, csrct=csrct, cdstt=cdstt,
                          ranksT=ranksT, dstmap=dstmap))
    return cores


# ------------------------------------------------------- multi-wait splitting
def _split_multiwaits(nc):
    """walrus in this stack accepts at most one sem-wait per instruction; Tile
    emits multi-wait instructions. Split extras into single-wait NoOps."""
    n = 0
    for fn in nc.m.functions:
        for blk in fn.blocks:
            out = []
            for inst in blk.instructions:
                si = inst.sync_info
                if si is not None and si.on_wait and len(si.on_wait) > 1:
                    waits = list(si.on_wait)
                    for i, w in enumerate(waits[:-1]):
                        nop = mybir.InstNoOp(name=f"{inst.name}_wsplit{i}", ins=[], outs=[])
                        nop.engine = inst.engine
                        nop.sync_info = mybir.SyncInfo(on_wait=[w], on_update=[])
                        out.append(nop)
                        n += 1
                    si.on_wait = [waits[-1]]
                out.append(inst)
            blk.instructions = out
    return n


# ----------------------------------------------------------- device program
def _build_program(nchunk, sigma):
    nt = nchunk // CPT
    epc = nchunk * P
    nc = bass.Bass(target_bir_lowering=False)

    din = {}
    for nm, shp, dt in [
        ("w1s", [256, HIDDEN], DT_BF), ("w2s", [HIDDEN, 2 * HIDDEN], DT_BF),
        ("w3s", [2 * HIDDEN, HIDDEN], DT_BF), ("w4p", [HIDDEN, P], DT_BF),
        ("b1s", [P, 8], DT_F32), ("b2s", [P, 16], DT_F32), ("b3s", [P, 8], DT_F32),
        ("b4bc", [P, P], DT_F32),
        ("w1d", [ATOM_EMBED, HIDDEN], DT_BF), ("w2d", [HIDDEN, 2 * HIDDEN], DT_BF),
        ("w3d", [2 * HIDDEN, HIDDEN], DT_BF), ("w4q", [HIDDEN, P], DT_BF),
        ("b1d", [P, 8], DT_F32), ("b2d", [P, 16], DT_F32), ("b3d", [P, 8], DT_F32),
        ("cvec", [P, 1], DT_F32), ("embt", [ATOM_EMBED, P], DT_BF),
        ("mubias", [P, 2], DT_F32), ("ones3", [3, 1], DT_F32),
        ("onesk1", [1, P], DT_F32), ("iota", [P, P], DT_F32),
        ("g1t", [HIDDEN, epc], DT_BF),
        ("csrct", [3, epc], DT_F32), ("cdstt", [3, epc], DT_F32),
        ("ranksT", [P, nchunk], DT_F32),
    ]:
        din[nm] = nc.declare_dram_parameter(nm, shp, dt, isOutput=False)

    zraw_d = nc.declare_dram_parameter("zraw", [nchunk, P, P], DT_F32, isOutput=True)
    dsttab_d = nc.declare_dram_parameter("dsttab", [P, P], DT_F32, isOutput=True)

    inv_s2 = float(1.0 / (sigma * sigma))

    with tile.TileContext(nc) as tc:
        with tc.tile_pool(name="wpool", bufs=1) as wp, \
             tc.tile_pool(name="cpool", bufs=1) as cp:
            C = {}
            for nm in ["b1s", "b2s", "b3s", "b4bc", "b1d", "b2d", "b3d",
                       "cvec", "mubias", "ones3", "onesk1", "iota"]:
                t = cp.tile(list(din[nm].shape), din[nm].dtype, tag=nm)
                nc.sync.dma_start(t[:], din[nm][:])
                C[nm] = t

            # ---------------- phase A: dst-table chain over 128 type columns
            with tc.tile_pool(name="apool", bufs=1) as ap, \
                 tc.tile_pool(name="apsum", bufs=4, space="PSUM") as aps:
                wd = {}
                for nm, ko, m in [("w1d", 2, HIDDEN), ("w2d", 8, 2 * HIDDEN),
                                  ("w3d", 16, HIDDEN), ("w4q", 8, P)]:
                    t = ap.tile([P, ko, m], DT_BF, tag=nm)
                    nc.sync.dma_start(t[:], din[nm].rearrange("(ko p) m -> p ko m", p=P))
                    wd[nm] = t
                embt = ap.tile([P, 2, P], DT_BF, tag="embt")
                nc.sync.dma_start(embt[:], din["embt"].rearrange("(ko p) m -> p ko m", p=P))

                def dense_a(win, kin, xin, mout, bias, out_tile):
                    for m in range(mout):
                        ps = aps.tile([P, P], DT_F32, tag="aps")
                        for k in range(kin):
                            nc.tensor.matmul(out=ps[:], lhsT=win[:, k, ts(m, P)],
                                             rhs=xin[:, k, :],
                                             start=(k == 0), stop=(k == kin - 1))
                        nc.scalar.activation(out_tile[:, m, :], ps[:],
                                             AF.Gelu_apprx_tanh, bias=bias[:, m:m + 1])

                h1d = ap.tile([P, 8, P], DT_BF, tag="h1d")
                dense_a(wd["w1d"], 2, embt, 8, C["b1d"], h1d)
                h2d = ap.tile([P, 16, P], DT_BF, tag="h2d")
                dense_a(wd["w2d"], 8, h1d, 16, C["b2d"], h2d)
                h3d = ap.tile([P, 8, P], DT_BF, tag="h3d")
                dense_a(wd["w3d"], 16, h2d, 8, C["b3d"], h3d)
                ps4 = aps.tile([P, P], DT_F32, tag="aps")
                for k in range(8):
                    nc.tensor.matmul(out=ps4[:], lhsT=wd["w4q"][:, k, :],
                                     rhs=h3d[:, k, :], start=(k == 0), stop=(k == 7))
                dtab = ap.tile([P, P], DT_F32, tag="dtab")
                nc.scalar.activation(dtab[:], ps4[:], AF.Identity, bias=C["cvec"][:])
                nc.sync.dma_start(dsttab_d[:], dtab[:])

            w1s = wp.tile([P, 2, HIDDEN], DT_BF, tag="w1s")
            w1v = din["w1s"].rearrange("(ko p) m -> p ko m", p=P)
            for k in range(2):
                nc.sync.dma_start(w1s[:, k, :], w1v[:, k, :])
            w2s = wp.tile([P, 8, 2 * HIDDEN], DT_BF, tag="w2s")
            w2v = din["w2s"].rearrange("(ko p) m -> p ko m", p=P)
            for k in range(8):
                nc.sync.dma_start(w2s[:, k, :], w2v[:, k, :])
            w3s = wp.tile([P, 16, HIDDEN], DT_BF, tag="w3s")
            w3v = din["w3s"].rearrange("(ko p) m -> p ko m", p=P)
            for k in range(16):
                nc.sync.dma_start(w3s[:, k, :], w3v[:, k, :])
            w4p = wp.tile([P, 8, P], DT_BF, tag="w4p")
            nc.sync.dma_start(w4p[:], din["w4p"].rearrange("(ko p) m -> p ko m", p=P))

            # ---------------- phase B: edge chain ---------------------------
            g1_v = din["g1t"].rearrange("(mo p) e -> p mo e", p=P)
            with tc.tile_pool(name="io", bufs=3) as io, \
                 tc.tile_pool(name="rb", bufs=2) as rb, \
                 tc.tile_pool(name="hp", bufs=2) as hp, \
                 tc.tile_pool(name="sp", bufs=3) as sp, \
                 tc.tile_pool(name="psA", bufs=3, space="PSUM") as psA, \
                 tc.tile_pool(name="psB", bufs=1, space="PSUM") as psB:

                def layer(win, kin, rhs_aps, mout, bias, out_tile):
                    for m in range(mout):
                        ps = psA.tile([P, T_EDGE], DT_F32, tag="mlp")
                        for k in range(kin):
                            nc.tensor.matmul(out=ps[:], lhsT=win[:, k, ts(m, P)],
                                             rhs=rhs_aps[k],
                                             start=(k == 0), stop=(k == kin - 1))
                        nc.scalar.activation(out_tile[:, m, :], ps[:],
                                             AF.Gelu_apprx_tanh, bias=bias[:, m:m + 1])

                for t in range(nt):
                    esl = ts(t, T_EDGE)
                    g1_sb = io.tile([P, 8, T_EDGE], DT_BF, tag="g1")
                    nc.sync.dma_start(g1_sb[:], g1_v[:, :, esl])
                    csrc = io.tile([3, T_EDGE], DT_F32, tag="csrc")
                    nc.sync.dma_start(csrc[:], din["csrct"][:, esl])
                    cdst = io.tile([3, T_EDGE], DT_F32, tag="cdst")
                    nc.sync.dma_start(cdst[:], din["cdstt"][:, esl])
                    rks = io.tile([P, CPT], DT_F32, tag="rks")
                    nc.sync.dma_start(rks[:], din["ranksT"][:, ts(t, CPT)])

                    # RBF edge_attr
                    diff = rb.tile([3, T_EDGE], DT_F32, tag="diff")
                    nc.vector.tensor_tensor(diff[:], csrc[:], cdst[:], op=ALU.subtract)
                    sq = rb.tile([3, T_EDGE], DT_F32, tag="sq")
                    nc.vector.tensor_tensor(sq[:], diff[:], diff[:], op=ALU.mult)
                    d2 = psB.tile([1, T_EDGE], DT_F32, tag="dzb", bufs=2, padded_shape=[P, T_EDGE])
                    nc.tensor.matmul(out=d2[:], lhsT=C["ones3"][:], rhs=sq[:],
                                     start=True, stop=True)
                    dsc = rb.tile([1, T_EDGE], DT_F32, tag="dsc")
                    nc.scalar.activation(dsc[:], d2[:], AF.Sqrt, scale=inv_s2)
                    rbf = rb.tile([P, 2, T_EDGE], DT_BF, tag="rbf")
                    for r in range(2):
                        zb = psB.tile([P, T_EDGE], DT_F32, tag="dzb", bufs=2)
                        nc.tensor.matmul(out=zb[:], lhsT=C["onesk1"][:], rhs=dsc[:],
                                         start=True, stop=True)
                        # z = d/sigma - mu/sigma ; z2 = z*z on DVE (keeps ACT
                        # table swaps down to Sqrt/Exp/Gelu)
                        zt = rb.tile([P, T_EDGE], DT_F32, tag="zt")
                        nc.vector.tensor_tensor(
                            zt[:], zb[:], C["mubias"][:, r:r + 1].to_broadcast([P, T_EDGE]),
                            op=ALU.add)
                        z2 = rb.tile([P, T_EDGE], DT_F32, tag="z2")
                        nc.vector.tensor_tensor(z2[:], zt[:], zt[:], op=ALU.mult)
                        nc.scalar.activation(rbf[:, r, :], z2[:], AF.Exp, scale=-0.5)

                    # MLP chain (feature-major); L1 = rbf matmul + gathered
                    # attr-table contribution (DVE add) + bias/gelu on ACT
                    h1 = hp.tile([P, 8, T_EDGE], DT_BF, tag="h1")
                    for m in range(8):
                        ps = psA.tile([P, T_EDGE], DT_F32, tag="mlp")
                        for k in range(2):
                            nc.tensor.matmul(out=ps[:], lhsT=w1s[:, k, ts(m, P)],
                                             rhs=rbf[:, k, :],
                                             start=(k == 0), stop=(k == 1))
                        tmp1 = rb.tile([P, T_EDGE], DT_F32, tag="tmp1")
                        nc.vector.tensor_tensor(tmp1[:], ps[:], g1_sb[:, m, :],
                                                op=ALU.add)
                        nc.scalar.activation(h1[:, m, :], tmp1[:],
                                             AF.Gelu_apprx_tanh,
                                             bias=C["b1s"][:, m:m + 1])
                    h2 = hp.tile([P, 16, T_EDGE], DT_BF, tag="h2")
                    layer(w2s, 8, [h1[:, k, :] for k in range(8)], 16, C["b2s"], h2)
                    h3 = hp.tile([P, 8, T_EDGE], DT_BF, tag="h3")
                    layer(w3s, 16, [h2[:, k, :] for k in range(16)], 8, C["b3s"], h3)

                    # layer 4 (edge-major) + rank-compressed segment sum
                    for cc in range(CPT):
                        psy = psB.tile([P, P], DT_F32, tag="l4", bufs=3)
                        for k in range(8):
                            nc.tensor.matmul(out=psy[:], lhsT=h3[:, k, ts(cc, P)],
                                             rhs=w4p[:, k, :],
                                             start=(k == 0), stop=(k == 7))
                        y = sp.tile([P, P], DT_F32, tag="y")
                        nc.vector.tensor_copy(y[:], psy[:])
                        S = sp.tile([P, P], DT_F32, tag="S")
                        nc.vector.tensor_tensor(
                            S[:], rks[:, cc:cc + 1].to_broadcast([P, P]), C["iota"][:],
                            op=ALU.is_equal)
                        psz = psB.tile([P, P], DT_F32, tag="l4", bufs=3)
                        nc.tensor.matmul(out=psz[:], lhsT=S[:], rhs=y[:],
                                         start=True, stop=False)
                        nc.tensor.matmul(out=psz[:], lhsT=S[:], rhs=C["b4bc"][:],
                                         start=False, stop=True)
                        z = sp.tile([P, P], DT_F32, tag="z")
                        nc.vector.tensor_copy(z[:], psz[:])
                        nc.sync.dma_start(zraw_d[t * CPT + cc], z[:])

    _split_multiwaits(nc)
    return nc


_PROG_CACHE = {}


def _get_program(nchunk, sigma):
    key = (nchunk, float(sigma))
    if key not in _PROG_CACHE:
        _PROG_CACHE[key] = _build_program(nchunk, sigma)
    return _PROG_CACHE[key]


def _assemble(atom_types, dsttab, core_Z, core_dstmaps):
    out = dsttab.T[np.asarray(atom_types)].astype(F32).copy()
    for Z, dstmap in zip(core_Z, core_dstmaps):
        flat = Z.reshape(-1, N_GAUSS)
        dm = dstmap.reshape(-1)
        valid = dm >= 0
        np.add.at(out, dm[valid], flat[valid].astype(F32))
    return out


def run_gcn(atom_types, atom_coord, edge_index, params, trace=False, **run_kwargs):
    W, emb, sigma, tab1 = _prep_constants(params)
    E = np.asarray(edge_index).shape[1]
    nchunk = _nchunk_for(E)
    cores = _prep_edges(atom_types, atom_coord, edge_index, tab1, nchunk)
    nc = _get_program(nchunk, sigma)

    in_maps = []
    for c in range(N_CORES):
        m = dict(W)
        m.pop("iota_np", None)
        cd = cores[c]
        m.update(g1t=cd["g1t"], csrct=cd["csrct"], cdstt=cd["cdstt"],
                 ranksT=cd["ranksT"])
        in_maps.append(m)

    res = run_bass_kernel_spmd(nc, in_maps, core_ids=list(range(N_CORES)),
                               trace=trace, **run_kwargs)
    core_Z = [res.results[c]["zraw"] for c in range(N_CORES)]
    dsttab = res.results[0]["dsttab"]
    out = _assemble(atom_types, dsttab, core_Z, [cd["dstmap"] for cd in cores])
    return out, res


def kernel(atom_types, atom_coord, edge_index, params):
    out, _ = run_gcn(atom_types, atom_coord, edge_index, params, trace=False)
    return out


# revision 13
# speedup vs baseline: 2.1963x; 2.1963x over previous
"""GCN encoder (nn_GCNEncoder_74990128988468) on 8 Trainium2 NeuronCores.

Strategy (edge-parallel, per the sharding hint):
  * Sort edges by dst, shard E/8 contiguous edges per core.
  * Per core the device runs the src_fc MLP (effective 512->1024->2048->1024->128;
    the node_feat block of the input is structurally zero) over its edges in
    bf16 with fp32 PSUM accumulation, computes the Gaussian radial edge_attr on
    device, and segment-sums via a rank-compression matmul per 128-edge chunk
    (S built on device with is_equal against an iota tile; fc's Wc slice is
    folded into src_fc layer 4).
  * dst_fc has only 100 distinct inputs (embedding rows) -> computed once per
    core as a 128-wide table; feat_fc(0) and all bias paths constant-folded
    into a single per-feature bias.
  * Host does index manipulation only: sort/shard/gather + final scatter-add.
"""
import sys

sys.path.insert(0, '/opt/trn_rl_repo')

import numpy as np
import ml_dtypes

import concourse.bass as bass
import concourse.mybir as mybir
import concourse.tile as tile
from concourse.bass_utils import run_bass_kernel_spmd

F32 = np.float32
BF16 = ml_dtypes.bfloat16

N_ATOM_TYPE = 100
N_GAUSS = 128
ATOM_EMBED = 256
RADIAL = 256
HIDDEN = 1024
CUTOFF = 5.0
N_CORES = 8
P = 128
T_EDGE = 512
CPT = T_EDGE // P              # chunks per compute tile

DT_BF = mybir.dt.bfloat16
DT_F32 = mybir.dt.float32
AF = mybir.ActivationFunctionType
ALU = mybir.AluOpType
ts = bass.ts


def _nchunk_for(E):
    epc = E // N_CORES
    nch = -(-epc // P)
    return -(-nch // CPT) * CPT


# ----------------------------------------------------------------- host math
def _gelu_tanh(x):
    x = x.astype(F32)
    return F32(0.5) * x * (F32(1.0) + np.tanh(
        F32(np.sqrt(2.0 / np.pi)) * (x + F32(0.044715) * x * x * x)))


def _mlp_host(ps, x):
    x = x.astype(F32)
    for i, (W, b) in enumerate(ps):
        x = x @ np.asarray(W, F32) + np.asarray(b, F32)
        if i < len(ps) - 1:
            x = _gelu_tanh(x)
    return x


def _prep_constants(params):
    emb = np.asarray(params["atom_embedding"], F32)
    src_ps = [(np.asarray(W, F32), np.asarray(b, F32)) for W, b in params["src_fc"]]
    dst_ps = [(np.asarray(W, F32), np.asarray(b, F32)) for W, b in params["dst_fc"]]
    feat_ps = [(np.asarray(W, F32), np.asarray(b, F32)) for W, b in params["feat_fc"]]
    Wfc = np.asarray(params["fc"][0], F32)
    bfc = np.asarray(params["fc"][1], F32)
    Wa, Wb, Wc = Wfc[0:128], Wfc[128:256], Wfc[256:384]

    r = _mlp_host(feat_ps, np.zeros((1, N_GAUSS), F32))[0]   # feat_fc(0)

    # L1 split: node_feat rows (0:128) multiply zeros -> dropped; node_attr
    # rows (128:384) have only 100 distinct inputs -> precompute emb @ W1a as
    # a [100, 1024] table (gathered per edge on host); only the radial rows
    # (384:640) stay as a matmul.
    W1a = src_ps[0][0][128:384]
    W1s = src_ps[0][0][384:640]
    tab1 = (emb @ W1a).astype(BF16)                          # [100, 1024]
    W4p = src_ps[3][0] @ Wc
    b4p = src_ps[3][1] @ Wc
    W4q = dst_ps[3][0] @ Wa
    cvec = dst_ps[3][1] @ Wa + r @ Wb + bfc

    embT = np.zeros((ATOM_EMBED, P), F32)
    embT[:, :N_ATOM_TYPE] = emb.T

    mu = np.linspace(0.0, CUTOFF, RADIAL, dtype=F32)
    sigma = F32(mu[1] - mu[0])

    def bfw(a):
        return np.ascontiguousarray(a.astype(BF16))

    def btile(b, nm):  # [nm*128] -> [128, nm]
        return np.ascontiguousarray(b.reshape(nm, P).T.astype(F32))

    W = dict(
        w1s=bfw(W1s), w2s=bfw(src_ps[1][0]), w3s=bfw(src_ps[2][0]), w4p=bfw(W4p),
        b1s=btile(src_ps[0][1], 8), b2s=btile(src_ps[1][1], 16), b3s=btile(src_ps[2][1], 8),
        b4bc=np.ascontiguousarray(np.broadcast_to(b4p.astype(F32), (P, P))),
        w1d=bfw(dst_ps[0][0]), w2d=bfw(dst_ps[1][0]), w3d=bfw(dst_ps[2][0]), w4q=bfw(W4q),
        b1d=btile(dst_ps[0][1], 8), b2d=btile(dst_ps[1][1], 16), b3d=btile(dst_ps[2][1], 8),
        cvec=np.ascontiguousarray(cvec.astype(F32)[:, None]),
        embt=bfw(embT),
        mubias=np.ascontiguousarray((-(mu / sigma)).reshape(2, P).T.astype(F32)),
        ones3=np.ones((3, 1), F32),
        onesk1=np.ones((1, P), F32),
        iota=np.ascontiguousarray(np.broadcast_to(np.arange(P, dtype=F32), (P, P))),
    )
    return W, emb, sigma, tab1


def _prep_edges(atom_types, atom_coord, edge_index, tab1, nchunk):
    atom_types = np.asarray(atom_types)
    atom_coord = np.asarray(atom_coord, F32)
    src = np.asarray(edge_index[0]).astype(np.int64)
    dst = np.asarray(edge_index[1]).astype(np.int64)
    E = src.shape[0]
    epc = E // N_CORES
    epc_pad = nchunk * P

    order = np.argsort(dst, kind="stable")
    src_s, dst_s = src[order], dst[order]

    cores = []
    for c in range(N_CORES):
        s = src_s[c * epc:(c + 1) * epc]
        d = dst_s[c * epc:(c + 1) * epc]
        pad = epc_pad - epc
        s_pad = np.concatenate([s, np.full(pad, s[0], np.int64)])
        d_pad = np.concatenate([d, np.full(pad, -1, np.int64)])

        is_new = np.ones(epc_pad, bool)
        is_new[1:] = d_pad[1:] != d_pad[:-1]
        is_new[::P] = True
        rank = (np.cumsum(is_new.reshape(nchunk, P).astype(np.int64), axis=1) - 1).reshape(-1)

        real = np.arange(epc_pad) < epc
        ranks_f = np.where(real, rank, -1).astype(F32)
        ranksT = np.ascontiguousarray(ranks_f.reshape(nchunk, P).T)  # [128, nchunk]

        dstmap = np.full((nchunk, P), -1, np.int64)
        fn = is_new & real
        cid = np.arange(epc_pad) // P
        dstmap[cid[fn], rank[fn]] = d_pad[fn]

        types_src = atom_types[s_pad]
        g1t = np.ascontiguousarray(tab1[types_src].T)                # [1024, epc_pad]
        csrct = np.ascontiguousarray(atom_coord[s_pad].T)            # [3, epc_pad]
        d_safe = np.where(real, np.maximum(d_pad, 0), s_pad)
        cdstt = np.ascontiguousarray(atom_coord[d_safe].T)

        cores.append(dict(g1t=g1t, csrct=csrct, cdstt=cdstt,
                          ranksT=ranksT, dstmap=dstmap))
    return cores


# ------------------------------------------------------- multi-wait splitting
def _split_multiwaits(nc):
    """walrus in this stack accepts at most one sem-wait per instruction; Tile
    emits multi-wait instructions. Split extras into single-wait NoOps."""
    n = 0
    for fn in nc.m.functions:
        for blk in fn.blocks:
            out = []
            for inst in blk.instructions:
                si = inst.sync_info
                if si is not None and si.on_wait and len(si.on_wait) > 1:
                    waits = list(si.on_wait)
                    for i, w in enumerate(waits[:-1]):
                        nop = mybir.InstNoOp(name=f"{inst.name}_wsplit{i}", ins=[], outs=[])
                        nop.engine = inst.engine
                        nop.sync_info = mybir.SyncInfo(on_wait=[w], on_update=[])
                        out.append(nop)
                        n += 1
                    si.on_wait = [waits[-1]]
                out.append(inst)
            blk.instructions = out
    return n


# ----------------------------------------------------------- device program
def _build_program(nchunk, sigma):
    nt = nchunk // CPT
    epc = nchunk * P
    nc = bass.Bass(target_bir_lowering=False)

    din = {}
    for nm, shp, dt in [
        ("w1s", [256, HIDDEN], DT_BF), ("w2s", [HIDDEN, 2 * HIDDEN], DT_BF),
        ("w3s", [2 * HIDDEN, HIDDEN], DT_BF), ("w4p", [HIDDEN, P], DT_BF),
        ("b1s", [P, 8], DT_F32), ("b2s", [P, 16], DT_F32), ("b3s", [P, 8], DT_F32),
        ("b4bc", [P, P], DT_F32),
        ("w1d", [ATOM_EMBED, HIDDEN], DT_BF), ("w2d", [HIDDEN, 2 * HIDDEN], DT_BF),
        ("w3d", [2 * HIDDEN, HIDDEN], DT_BF), ("w4q", [HIDDEN, P], DT_BF),
        ("b1d", [P, 8], DT_F32), ("b2d", [P, 16], DT_F32), ("b3d", [P, 8], DT_F32),
        ("cvec", [P, 1], DT_F32), ("embt", [ATOM_EMBED, P], DT_BF),
        ("mubias", [P, 2], DT_F32), ("ones3", [3, 1], DT_F32),
        ("onesk1", [1, P], DT_F32), ("iota", [P, P], DT_F32),
        ("g1t", [HIDDEN, epc], DT_BF),
        ("csrct", [3, epc], DT_F32), ("cdstt", [3, epc], DT_F32),
        ("ranksT", [P, nchunk], DT_F32),
    ]:
        din[nm] = nc.declare_dram_parameter(nm, shp, dt, isOutput=False)

    zraw_d = nc.declare_dram_parameter("zraw", [nchunk, P, P], DT_F32, isOutput=True)
    dsttab_d = nc.declare_dram_parameter("dsttab", [P, P], DT_F32, isOutput=True)

    inv_s2 = float(1.0 / (sigma * sigma))

    with tile.TileContext(nc) as tc:
        with tc.tile_pool(name="wpool", bufs=1) as wp, \
             tc.tile_pool(name="cpool", bufs=1) as cp:
            C = {}
            for nm in ["b1s", "b2s", "b3s", "b4bc", "b1d", "b2d", "b3d",
                       "cvec", "mubias", "ones3", "onesk1", "iota"]:
                t = cp.tile(list(din[nm].shape), din[nm].dtype, tag=nm)
                nc.sync.dma_start(t[:], din[nm][:])
                C[nm] = t

            # ---------------- phase A: dst-table chain over 128 type columns
            with tc.tile_pool(name="apool", bufs=1) as ap, \
                 tc.tile_pool(name="apsum", bufs=4, space="PSUM") as aps:
                wd = {}
                for nm, ko, m in [("w1d", 2, HIDDEN), ("w2d", 8, 2 * HIDDEN),
                                  ("w3d", 16, HIDDEN), ("w4q", 8, P)]:
                    t = ap.tile([P, ko, m], DT_BF, tag=nm)
                    nc.sync.dma_start(t[:], din[nm].rearrange("(ko p) m -> p ko m", p=P))
                    wd[nm] = t
                embt = ap.tile([P, 2, P], DT_BF, tag="embt")
                nc.sync.dma_start(embt[:], din["embt"].rearrange("(ko p) m -> p ko m", p=P))

                def dense_a(win, kin, xin, mout, bias, out_tile):
                    for m in range(mout):
                        ps = aps.tile([P, P], DT_F32, tag="aps")
                        for k in range(kin):
                            nc.tensor.matmul(out=ps[:], lhsT=win[:, k, ts(m, P)],
                                             rhs=xin[:, k, :],
                                             start=(k == 0), stop=(k == kin - 1))
                        nc.scalar.activation(out_tile[:, m, :], ps[:],
                                             AF.Gelu_apprx_tanh, bias=bias[:, m:m + 1])

                h1d = ap.tile([P, 8, P], DT_BF, tag="h1d")
                dense_a(wd["w1d"], 2, embt, 8, C["b1d"], h1d)
                h2d = ap.tile([P, 16, P], DT_BF, tag="h2d")
                dense_a(wd["w2d"], 8, h1d, 16, C["b2d"], h2d)
                h3d = ap.tile([P, 8, P], DT_BF, tag="h3d")
                dense_a(wd["w3d"], 16, h2d, 8, C["b3d"], h3d)
                ps4 = aps.tile([P, P], DT_F32, tag="aps")
                for k in range(8):
                    nc.tensor.matmul(out=ps4[:], lhsT=wd["w4q"][:, k, :],
                                     rhs=h3d[:, k, :], start=(k == 0), stop=(k == 7))
                dtab = ap.tile([P, P], DT_F32, tag="dtab")
                nc.scalar.activation(dtab[:], ps4[:], AF.Identity, bias=C["cvec"][:])
                nc.sync.dma_start(dsttab_d[:], dtab[:])

            w1s = wp.tile([P, 2, HIDDEN], DT_BF, tag="w1s")
            w1v = din["w1s"].rearrange("(ko p) m -> p ko m", p=P)
            for k in range(2):
                nc.sync.dma_start(w1s[:, k, :], w1v[:, k, :])
            w2s = wp.tile([P, 8, 2 * HIDDEN], DT_BF, tag="w2s")
            w2v = din["w2s"].rearrange("(ko p) m -> p ko m", p=P)
            for k in range(8):
                nc.sync.dma_start(w2s[:, k, :], w2v[:, k, :])
            w3s = wp.tile([P, 16, HIDDEN], DT_BF, tag="w3s")
            w3v = din["w3s"].rearrange("(ko p) m -> p ko m", p=P)
            for k in range(16):
                nc.sync.dma_start(w3s[:, k, :], w3v[:, k, :])
            w4p = wp.tile([P, 8, P], DT_BF, tag="w4p")
            nc.sync.dma_start(w4p[:], din["w4p"].rearrange("(ko p) m -> p ko m", p=P))

            # ---------------- phase B: edge chain ---------------------------
            g1_v = din["g1t"].rearrange("(mo p) e -> p mo e", p=P)
            with tc.tile_pool(name="io", bufs=3) as io, \
                 tc.tile_pool(name="rb", bufs=2) as rb, \
                 tc.tile_pool(name="hp", bufs=2) as hp, \
                 tc.tile_pool(name="sp", bufs=3) as sp, \
                 tc.tile_pool(name="psA", bufs=3, space="PSUM") as psA, \
                 tc.tile_pool(name="psB", bufs=1, space="PSUM") as psB:

                def layer(win, kin, rhs_aps, mout, bias, out_tile):
                    for m in range(mout):
                        ps = psA.tile([P, T_EDGE], DT_F32, tag="mlp")
                        for k in range(kin):
                            nc.tensor.matmul(out=ps[:], lhsT=win[:, k, ts(m, P)],
                                             rhs=rhs_aps[k],
                                             start=(k == 0), stop=(k == kin - 1))
                        nc.scalar.activation(out_tile[:, m, :], ps[:],
                                             AF.Gelu_apprx_tanh, bias=bias[:, m:m + 1])

                pending_seg = None
                for t in range(nt):
                    esl = ts(t, T_EDGE)
                    g1_sb = io.tile([P, 8, T_EDGE], DT_BF, tag="g1")
                    nc.sync.dma_start(g1_sb[:], g1_v[:, :, esl])
                    csrc = io.tile([3, T_EDGE], DT_F32, tag="csrc")
                    nc.sync.dma_start(csrc[:], din["csrct"][:, esl])
                    cdst = io.tile([3, T_EDGE], DT_F32, tag="cdst")
                    nc.sync.dma_start(cdst[:], din["cdstt"][:, esl])
                    rks = io.tile([P, CPT], DT_F32, tag="rks")
                    nc.sync.dma_start(rks[:], din["ranksT"][:, ts(t, CPT)])

                    # RBF edge_attr
                    diff = rb.tile([3, T_EDGE], DT_F32, tag="diff")
                    nc.vector.tensor_tensor(diff[:], csrc[:], cdst[:], op=ALU.subtract)
                    sq = rb.tile([3, T_EDGE], DT_F32, tag="sq")
                    nc.vector.tensor_tensor(sq[:], diff[:], diff[:], op=ALU.mult)
                    d2 = psB.tile([1, T_EDGE], DT_F32, tag="dzb", bufs=2, padded_shape=[P, T_EDGE])
                    nc.tensor.matmul(out=d2[:], lhsT=C["ones3"][:], rhs=sq[:],
                                     start=True, stop=True)
                    dsc = rb.tile([1, T_EDGE], DT_F32, tag="dsc")
                    nc.scalar.activation(dsc[:], d2[:], AF.Sqrt, scale=inv_s2)
                    rbf = rb.tile([P, 2, T_EDGE], DT_BF, tag="rbf")
                    for r in range(2):
                        zb = psB.tile([P, T_EDGE], DT_F32, tag="dzb", bufs=2)
                        nc.tensor.matmul(out=zb[:], lhsT=C["onesk1"][:], rhs=dsc[:],
                                         start=True, stop=True)
                        # z = d/sigma - mu/sigma ; z2 = z*z on DVE (keeps ACT
                        # table swaps down to Sqrt/Exp/Gelu)
                        zt = rb.tile([P, T_EDGE], DT_F32, tag="zt")
                        nc.vector.tensor_tensor(
                            zt[:], zb[:], C["mubias"][:, r:r + 1].to_broadcast([P, T_EDGE]),
                            op=ALU.add)
                        z2 = rb.tile([P, T_EDGE], DT_F32, tag="z2")
                        nc.vector.tensor_tensor(z2[:], zt[:], zt[:], op=ALU.mult)
                        nc.scalar.activation(rbf[:, r, :], z2[:], AF.Exp, scale=-0.5)

                    # MLP chain (feature-major); L1 = rbf matmul + gathered
                    # attr-table contribution (DVE add) + bias/gelu on ACT
                    h1 = hp.tile([P, 8, T_EDGE], DT_BF, tag="h1")
                    for m in range(8):
                        ps = psA.tile([P, T_EDGE], DT_F32, tag="mlp")
                        for k in range(2):
                            nc.tensor.matmul(out=ps[:], lhsT=w1s[:, k, ts(m, P)],
                                             rhs=rbf[:, k, :],
                                             start=(k == 0), stop=(k == 1))
                        tmp1 = rb.tile([P, T_EDGE], DT_F32, tag="tmp1")
                        nc.vector.tensor_tensor(tmp1[:], ps[:], g1_sb[:, m, :],
                                                op=ALU.add)
                        nc.scalar.activation(h1[:, m, :], tmp1[:],
                                             AF.Gelu_apprx_tanh,
                                             bias=C["b1s"][:, m:m + 1])
                    if pending_seg is not None:
                        pending_seg()
                    h2 = hp.tile([P, 16, T_EDGE], DT_BF, tag="h2")
                    layer(w2s, 8, [h1[:, k, :] for k in range(8)], 16, C["b2s"], h2)
                    h3 = hp.tile([P, 8, T_EDGE], DT_BF, tag="h3")
                    layer(w3s, 16, [h2[:, k, :] for k in range(16)], 8, C["b3s"], h3)

                    # layer 4 (edge-major) + rank-compressed segment sum;
                    # deferred one tile so the chunk chains overlap the next
                    # tile's dense L2/L3 stream
                    def _emit_seg(h3=h3, rks=rks, t=t):
                        for cc in range(CPT):
                            psy = psB.tile([P, P], DT_F32, tag="l4", bufs=3)
                            for k in range(8):
                                nc.tensor.matmul(out=psy[:], lhsT=h3[:, k, ts(cc, P)],
                                                 rhs=w4p[:, k, :],
                                                 start=(k == 0), stop=(k == 7))
                            y = sp.tile([P, P], DT_F32, tag="y")
                            nc.vector.tensor_copy(y[:], psy[:])
                            S = sp.tile([P, P], DT_F32, tag="S")
                            nc.vector.tensor_tensor(
                                S[:], rks[:, cc:cc + 1].to_broadcast([P, P]), C["iota"][:],
                                op=ALU.is_equal)
                            psz = psB.tile([P, P], DT_F32, tag="l4", bufs=3)
                            nc.tensor.matmul(out=psz[:], lhsT=S[:], rhs=y[:],
                                             start=True, stop=False)
                            nc.tensor.matmul(out=psz[:], lhsT=S[:], rhs=C["b4bc"][:],
                                             start=False, stop=True)
                            z = sp.tile([P, P], DT_F32, tag="z")
                            nc.vector.tensor_copy(z[:], psz[:])
                            nc.sync.dma_start(zraw_d[t * CPT + cc], z[:])
                    pending_seg = _emit_seg
                if pending_seg is not None:
                    pending_seg()

    _split_multiwaits(nc)
    return nc


_PROG_CACHE = {}


def _get_program(nchunk, sigma):
    key = (nchunk, float(sigma))
    if key not in _PROG_CACHE:
        _PROG_CACHE[key] = _build_program(nchunk, sigma)
    return _PROG_CACHE[key]


def _assemble(atom_types, dsttab, core_Z, core_dstmaps):
    out = dsttab.T[np.asarray(atom_types)].astype(F32).copy()
    for Z, dstmap in zip(core_Z, core_dstmaps):
        flat = Z.reshape(-1, N_GAUSS)
        dm = dstmap.reshape(-1)
        valid = dm >= 0
        np.add.at(out, dm[valid], flat[valid].astype(F32))
    return out


def run_gcn(atom_types, atom_coord, edge_index, params, trace=False, **run_kwargs):
    W, emb, sigma, tab1 = _prep_constants(params)
    E = np.asarray(edge_index).shape[1]
    nchunk = _nchunk_for(E)
    cores = _prep_edges(atom_types, atom_coord, edge_index, tab1, nchunk)
    nc = _get_program(nchunk, sigma)

    in_maps = []
    for c in range(N_CORES):
        m = dict(W)
        m.pop("iota_np", None)
        cd = cores[c]
        m.update(g1t=cd["g1t"], csrct=cd["csrct"], cdstt=cd["cdstt"],
                 ranksT=cd["ranksT"])
        in_maps.append(m)

    res = run_bass_kernel_spmd(nc, in_maps, core_ids=list(range(N_CORES)),
                               trace=trace, **run_kwargs)
    core_Z = [res.results[c]["zraw"] for c in range(N_CORES)]
    dsttab = res.results[0]["dsttab"]
    out = _assemble(atom_types, dsttab, core_Z, [cd["dstmap"] for cd in cores])
    return out, res


def kernel(atom_types, atom_coord, edge_index, params):
    out, _ = run_gcn(atom_types, atom_coord, edge_index, params, trace=False)
    return out
